# revision 2
# baseline (speedup 1.0000x reference)
"""Trainium2 Bass kernel for nn_Erode (5x5 all-ones SE, zero padding).

For an all-ones 5x5 structuring element, kornia-style Erode reduces to a
5x5 sliding-window MIN over the zero-padded image.  The min is separable
and the minimal 2-input-op decomposition is 6 passes (provable):
  vertical   U[s]=min(X[s],X[s+3]); V[s]=min(U[s],U[s+1]); W[s]=min(V[s],X[s+2])
  horizontal T1[c]=min(W[c],W[c+3]); T2[c]=min(T1[c],T1[c+1]); H[c]=min(T2[c],W[c+2])
(40 + 3cw+1 rows per 13+cw outputs -- one row/col cheaper than the naive
pair/quad scheme).  All mins run on the DVE in fp16 at the 2x_1p rate
(~0.52 ns per free-dim elem); no other engine can run 2-tensor mins
(walrus ISA check rejects TensorTensor AND TensorScalarPtr on Pool; ACT
has no 2-tensor ops; DMA cce only does add).

Schedule (v2): chunks of columns sized (64,160,256,32) so that
 * a small-ish first chunk + slot-split input pieces on both HWDGE rings
   start the DVE early (U sub-ops chase arriving slot pieces),
 * each chunk's input lands before its Q/V/W passes need it
   (cumulative DVE work per col ~41.6ns vs 2-ring arrival ~19.1ns),
 * stores are split per-chunk into pieces alternating rings so output
   streaming overlaps compute and the final piece is tiny (short drain).
Fixed costs measured on this silicon: ~1.1us preamble barrier, ~0.6us
DIRECT2D descriptor-gen per dma_start (128 descriptors), ~0.6us DMA
first-byte latency, ~8.5us NRT postamble semaphore cascade (immutable).
Input DMAs move only the 120 data-carrying partitions.

Distribution: pure data parallel.  B*C = 24 images of 512x512, 3 per
core across 8 NeuronCores.  Partition p = 40*i + j owns K=13 output
rows of image i as 17 slot rows (2+2 vertical halo), host-pre-gathered
so every DMA is contiguous per partition.
"""

import numpy as np

# ---- fixed problem geometry (hardcoded per harness contract) ----
B, C, H, W = 8, 3, 512, 512
N_CORES = 8
IMGS = (B * C) // N_CORES  # 3 images per core
K = 13                    # output rows per partition
SLOTS = K + 4             # slot rows incl. 2+2 halo
PPI = 40                  # partitions per image = ceil(512/13)
NPD = IMGS * PPI          # 120 data partitions
PAD_H = 2 + H + 12        # 526 padded rows (covers slot overrun)
PAD_W = 2 + W + 2         # 516 padded cols

# column chunks (c0, c1): growing sizes for DMA/compute pipelining,
# small last chunk for a short store drain.
CHUNKS = [(0, 64), (64, 224), (224, 480), (480, 512)]
LWS = [c1 - c0 + 4 for c0, c1 in CHUNKS]

# input slot pieces per chunk: (s0, s1, ring)  ring 0=sync 1=scalar.
# Fine pieces early so U sub-ops can chase arrivals; ring loads balanced.
IN_PIECES = [
    [(0, 5, 0), (5, 9, 1), (9, 13, 0), (13, 17, 1)],
    [(0, 5, 1), (5, 9, 0), (9, 13, 1), (13, 17, 0)],
    [(0, 9, 0), (9, 17, 1)],
    [(0, 9, 1), (9, 17, 0)],
]
# U pass split boundaries per chunk (rows of U, 14 total).  U[a:b] needs
# slots a..b+2, i.e. waits only on the input pieces covering them.
U_SPLITS = [[0, 2, 6, 14], [0, 6, 14], [0, 6, 14], [0, 14]]
# store pieces per chunk: (b0, b1, ring), cols relative to chunk
H_PIECES = [
    [(0, 64, 1)],
    [(0, 80, 0), (80, 160, 1)],
    [(0, 64, 0), (64, 128, 1), (128, 192, 0), (192, 256, 1)],
    [(0, 16, 0), (16, 32, 1)],
]

IN_ELEMS = NPD * SLOTS * sum(LWS)
OUT_ELEMS = NPD * K * W

_cached = {}


def _build_program():
    import concourse.mybir as mybir
    from concourse import bass, bacc
    from concourse.tile import TileContext

    f16 = mybir.dt.float16
    MIN = mybir.AluOpType.min

    nc = bacc.Bacc("TRN2", target_bir_lowering=False, debug=False,
                   num_devices=N_CORES)
    xs = nc.dram_tensor("xs", [IN_ELEMS], f16, kind="ExternalInput")
    ys = nc.dram_tensor("ys", [OUT_ELEMS], f16, kind="ExternalOutput")

    with TileContext(nc) as tc:
        with tc.tile_pool(name="work", bufs=1) as pool:
            rings = [nc.sync, nc.scalar]
            # issue ALL input DMAs up front (ring FIFO keeps input ahead
            # of stores; descriptor-gen of piece k+1 overlaps stream k).
            xt = []
            in_off = 0
            for ch in range(len(CHUNKS)):
                lw = LWS[ch]
                X = pool.tile([NPD, SLOTS, lw], f16, tag=f"X{ch}")
                for (s0, s1, r) in IN_PIECES[ch]:
                    src = bass.AP(
                        tensor=xs,
                        offset=in_off + s0 * lw,
                        ap=[[SLOTS * lw, NPD], [1, (s1 - s0) * lw]],
                    )
                    rings[r].dma_start(out=X[:, s0:s1], in_=src)
                in_off += NPD * SLOTS * lw
                xt.append(X)

            out_off = 0
            for ch, (c0, c1) in enumerate(CHUNKS):
                lw = LWS[ch]
                cw = c1 - c0
                X = xt[ch]

                # vertical 5-tap min along slot rows (all DVE 2x):
                # U={0,3}, V=U+{0,1} -> {0,1,3,4}, W=V+{2} -> {0..4}
                U = pool.tile([NPD, 14, lw], f16, tag=f"U{ch}")
                bs = U_SPLITS[ch]
                for a, b in zip(bs[:-1], bs[1:]):
                    nc.vector.tensor_tensor(out=U[:, a:b], in0=X[:, a:b],
                                            in1=X[:, a + 3:b + 3], op=MIN)
                V = pool.tile([NPD, K, lw], f16, tag=f"V{ch}")
                nc.vector.tensor_tensor(out=V, in0=U[:, 0:K],
                                        in1=U[:, 1:K + 1], op=MIN)
                Wt = pool.tile([NPD, K, lw], f16, tag=f"W{ch}")
                nc.vector.tensor_tensor(out=Wt, in0=V,
                                        in1=X[:, 2:K + 2], op=MIN)

                # horizontal 5-tap min along cols (same decomposition).
                # T1 padded to even row stride so fp16 rows stay 4B-aligned.
                T1 = pool.tile([NPD, K, cw + 2], f16, tag=f"T1{ch}")
                nc.vector.tensor_tensor(out=T1[:, :, 0:cw + 1],
                                        in0=Wt[:, :, 0:cw + 1],
                                        in1=Wt[:, :, 3:cw + 4], op=MIN)
                T2 = pool.tile([NPD, K, cw], f16, tag=f"T2{ch}")
                nc.vector.tensor_tensor(out=T2, in0=T1[:, :, 0:cw],
                                        in1=T1[:, :, 1:cw + 1], op=MIN)

                for (b0, b1, r) in H_PIECES[ch]:
                    pw = b1 - b0
                    Hm = pool.tile([NPD, K, pw], f16, tag=f"H{ch}_{b0}")
                    nc.vector.tensor_tensor(
                        out=Hm, in0=T2[:, :, b0:b1],
                        in1=Wt[:, :, b0 + 2:b1 + 2], op=MIN)
                    dst = bass.AP(
                        tensor=ys,
                        offset=out_off,
                        ap=[[K * pw, NPD], [1, K * pw]],
                    )
                    rings[r].dma_start(out=dst, in_=Hm)
                    out_off += NPD * K * pw
    nc.compile()
    return nc


def _get_program():
    if "nc" not in _cached:
        _cached["nc"] = _build_program()
    return _cached["nc"]


# stripe gather index: padded-row index per (j, s)
_ROW_IDX = (K * np.arange(PPI)[:, None] + np.arange(SLOTS)[None, :])


def _stripe_core_input(x3: np.ndarray) -> np.ndarray:
    """[3,512,512] f16 -> host-striped flat input (chunk-blocked)."""
    xp = np.zeros((IMGS, PAD_H, PAD_W), np.float16)
    xp[:, 2:2 + H, 2:2 + W] = x3
    stripes = xp[:, _ROW_IDX, :].reshape(NPD, SLOTS, PAD_W)
    parts = [
        np.ascontiguousarray(stripes[:, :, c0:c0 + lw]).reshape(-1)
        for (c0, _), lw in zip(CHUNKS, LWS)
    ]
    return np.concatenate(parts)


def _out_pieces():
    pieces = []
    for ch, (c0, c1) in enumerate(CHUNKS):
        for (b0, b1, _r) in H_PIECES[ch]:
            pieces.append((c0 + b0, b1 - b0))
    return pieces


_PIECES = None


def _unstripe_core_output(flat: np.ndarray) -> np.ndarray:
    """piece-blocked f16 output -> [3,512,512] f32."""
    global _PIECES
    if _PIECES is None:
        _PIECES = _out_pieces()
    stripes = np.empty((NPD, K, W), np.float16)
    off = 0
    for col0, pw in _PIECES:
        blk = flat[off:off + NPD * K * pw].reshape(NPD, K, pw)
        stripes[:, :, col0:col0 + pw] = blk
        off += NPD * K * pw
    ys = stripes.reshape(IMGS, PPI, K, W)
    out = np.empty((IMGS, H, W), np.float32)
    full = (PPI - 1) * K  # 507 rows from full partitions
    out[:, :full] = ys[:, :PPI - 1].reshape(IMGS, full, W)
    out[:, full:] = ys[:, PPI - 1, :H - full]
    return out


def _run_on_hw(x24: np.ndarray, trace: bool = False):
    from concourse.bass_utils import run_bass_kernel_spmd
    nc = _get_program()
    x24 = x24.astype(np.float16)
    in_maps = [
        {"xs": _stripe_core_input(x24[IMGS * k:IMGS * (k + 1)])}
        for k in range(N_CORES)
    ]
    try:
        res = run_bass_kernel_spmd(nc, in_maps, list(range(N_CORES)),
                                   trace=trace)
    except Exception:
        import time
        time.sleep(5)
        res = run_bass_kernel_spmd(nc, in_maps, list(range(N_CORES)),
                                   trace=trace)
    out = np.stack([
        _unstripe_core_output(res.results[k]["ys"]) for k in range(N_CORES)
    ])
    return out.reshape(B, C, H, W), res


def _erode_reference_np(x: np.ndarray, se: np.ndarray) -> np.ndarray:
    """Generic fallback faithful to the kornia-style formula (numpy)."""
    kh, kw = se.shape
    ph, pw = kh // 2, kw // 2
    xpad = np.pad(x, ((0, 0), (0, 0), (ph, ph), (pw, pw)))
    out = None
    for r in range(kh):
        for c in range(kw):
            shifted = xpad[:, :, r:r + x.shape[2], c:c + x.shape[3]]
            bias = se[r, c] - 1.0
            val = shifted - bias if bias >= 0.0 else np.full_like(shifted, -bias)
            out = val if out is None else np.minimum(out, val)
    return out.astype(x.dtype)


def kernel(x, se):
    x = np.asarray(x, dtype=np.float32)
    se = np.asarray(se, dtype=np.float32)
    if se.shape != (5, 5) or not np.all(se == 1.0) or x.shape != (B, C, H, W):
        return _erode_reference_np(x, se)
    x24 = np.ascontiguousarray(x.reshape(B * C, H, W))
    out, _ = _run_on_hw(x24, trace=False)
    return out


# revision 6
# speedup vs baseline: 1.0950x; 1.0950x over previous
"""Trainium2 Bass kernel for nn_Erode (5x5 all-ones SE, zero padding).

For an all-ones 5x5 structuring element, kornia-style Erode reduces to a
5x5 sliding-window MIN over the zero-padded image.  The min is separable
and the minimal 2-input-op decomposition is 6 passes (provable):
  vertical   U[s]=min(X[s],X[s+3]); V[s]=min(U[s],U[s+1]); W[s]=min(V[s],X[s+2])
  horizontal T1[c]=min(W[c],W[c+3]); T2[c]=min(T1[c],T1[c+1]); H[c]=min(T2[c],W[c+2])
(40 + 3cw+1 rows per 13+cw outputs -- one row/col cheaper than the naive
pair/quad scheme).  All mins run on the DVE in fp16 at the 2x_1p rate
(~0.52 ns per free-dim elem); no other engine can run 2-tensor mins
(walrus ISA check rejects TensorTensor AND TensorScalarPtr on Pool; ACT
has no 2-tensor ops; DMA cce only does add).

Schedule (v2): chunks of columns sized (64,160,256,32) so that
 * a small-ish first chunk + slot-split input pieces on both HWDGE rings
   start the DVE early (U sub-ops chase arriving slot pieces),
 * each chunk's input lands before its Q/V/W passes need it
   (cumulative DVE work per col ~41.6ns vs 2-ring arrival ~19.1ns),
 * stores are split per-chunk into pieces alternating rings so output
   streaming overlaps compute and the final piece is tiny (short drain).
Fixed costs measured on this silicon: ~1.1us preamble barrier, ~0.6us
DIRECT2D descriptor-gen per dma_start (128 descriptors), ~0.6us DMA
first-byte latency, ~8.5us NRT postamble semaphore cascade (immutable).
Input DMAs move only the 120 data-carrying partitions.

Distribution: pure data parallel.  B*C = 24 images of 512x512, 3 per
core across 8 NeuronCores.  Partition p = 40*i + j owns K=13 output
rows of image i as 17 slot rows (2+2 vertical halo), host-pre-gathered
so every DMA is contiguous per partition.
"""

import numpy as np

# ---- fixed problem geometry (hardcoded per harness contract) ----
B, C, H, W = 8, 3, 512, 512
N_CORES = 8
IMGS = (B * C) // N_CORES  # 3 images per core
K = 13                    # output rows per partition
SLOTS = K + 4             # slot rows incl. 2+2 halo
PPI = 40                  # partitions per image = ceil(512/13)
NPD = IMGS * PPI          # 120 data partitions
PAD_H = 2 + H + 12        # 526 padded rows (covers slot overrun)
PAD_W = 2 + W + 2         # 516 padded cols

# column chunks (c0, c1): growing sizes for DMA/compute pipelining,
# small last chunk for a short store drain.
#
# DMA model (hardware-measured): each descriptor costs ~14.3ns on the
# SDMA path, so a piece streams at max(n_desc*14.3ns, bytes/150B-per-ns)
# per ring.  Descriptors are per-partition runs -> keep pieces >= ~2KB
# per partition, and use PARTITION-split (not slot-split) pieces where
# latency matters: 60-descriptor halves on both rings land in ~0.9us
# instead of a 120-descriptor piece's ~1.7us floor.
CHUNKS = [(0, 48), (48, 208), (208, 464), (464, 512)]
LWS = [c1 - c0 + 4 for c0, c1 in CHUNKS]

# input pieces per chunk: ("p", p0, p1, ring) partition-split whole-slot
# piece, or ("s", s0, s1, ring) slot-split full-partition piece.
# ring 0=sync 1=scalar.  Chunk 0 is partition-split across both rings
# for the fastest possible first-compute; later chunks use big
# slot-split pieces (one per ring); the tiny last chunk rides one ring.
IN_PIECES = [
    [("p", 0, 60, 0), ("p", 60, 120, 1)],
    [("s", 0, 9, 1), ("s", 9, 17, 0)],
    [("s", 0, 9, 0), ("s", 9, 17, 1)],
    [("s", 0, 17, 1)],
]
# U pass split boundaries per chunk (rows of U, 14 total).  U[a:b] needs
# slots a..b+2, i.e. waits only on the input pieces covering them.
U_SPLITS = [[0, 14], [0, 6, 14], [0, 14], [0, 14]]
# store pieces per chunk: (b0, b1, ring, psplit) cols relative to chunk.
# psplit=True stores the piece as two partition-halves on BOTH rings
# (rings then means the first half's ring) -- used for the final piece
# to halve the end-of-kernel drain.
H_PIECES = [
    [(0, 48, 0, False)],
    [(0, 160, 1, False)],
    [(0, 128, 0, False), (128, 256, 1, False)],
    [(0, 48, 0, True)],
]

IN_ELEMS = NPD * SLOTS * sum(LWS)
OUT_ELEMS = NPD * K * W

_cached = {}


def _build_program():
    import concourse.mybir as mybir
    from concourse import bass, bacc
    from concourse.tile import TileContext

    f16 = mybir.dt.float16
    MIN = mybir.AluOpType.min

    nc = bacc.Bacc("TRN2", target_bir_lowering=False, debug=False,
                   num_devices=N_CORES)
    xs = nc.dram_tensor("xs", [IN_ELEMS], f16, kind="ExternalInput")
    ys = nc.dram_tensor("ys", [OUT_ELEMS], f16, kind="ExternalOutput")

    with TileContext(nc) as tc:
        with tc.tile_pool(name="work", bufs=1) as pool:
            rings = [nc.sync, nc.scalar]
            # issue ALL input DMAs up front (ring FIFO keeps input ahead
            # of stores; descriptor-gen of piece k+1 overlaps stream k).
            xt = []
            in_off = 0
            for ch in range(len(CHUNKS)):
                lw = LWS[ch]
                X = pool.tile([NPD, SLOTS, lw], f16, tag=f"X{ch}")
                for (kind, a, b, r) in IN_PIECES[ch]:
                    if kind == "p":   # partition range [a, b), all slots
                        src = bass.AP(
                            tensor=xs,
                            offset=in_off + a * SLOTS * lw,
                            ap=[[SLOTS * lw, b - a], [1, SLOTS * lw]],
                        )
                        rings[r].dma_start(out=X[a:b], in_=src)
                    else:             # slot range [a, b), all partitions
                        src = bass.AP(
                            tensor=xs,
                            offset=in_off + a * lw,
                            ap=[[SLOTS * lw, NPD], [1, (b - a) * lw]],
                        )
                        rings[r].dma_start(out=X[:, a:b], in_=src)
                in_off += NPD * SLOTS * lw
                xt.append(X)

            out_off = 0
            for ch, (c0, c1) in enumerate(CHUNKS):
                lw = LWS[ch]
                cw = c1 - c0
                X = xt[ch]

                # vertical 5-tap min along slot rows (all DVE 2x):
                # U={0,3}, V=U+{0,1} -> {0,1,3,4}, W=V+{2} -> {0..4}
                U = pool.tile([NPD, 14, lw], f16, tag=f"U{ch}")
                bs = U_SPLITS[ch]
                for a, b in zip(bs[:-1], bs[1:]):
                    nc.vector.tensor_tensor(out=U[:, a:b], in0=X[:, a:b],
                                            in1=X[:, a + 3:b + 3], op=MIN)
                V = pool.tile([NPD, K, lw], f16, tag=f"V{ch}")
                nc.vector.tensor_tensor(out=V, in0=U[:, 0:K],
                                        in1=U[:, 1:K + 1], op=MIN)
                Wt = pool.tile([NPD, K, lw], f16, tag=f"W{ch}")
                nc.vector.tensor_tensor(out=Wt, in0=V,
                                        in1=X[:, 2:K + 2], op=MIN)

                # horizontal 5-tap min along cols (same decomposition).
                # T1 padded to even row stride so fp16 rows stay 4B-aligned.
                T1 = pool.tile([NPD, K, cw + 2], f16, tag=f"T1{ch}")
                nc.vector.tensor_tensor(out=T1[:, :, 0:cw + 1],
                                        in0=Wt[:, :, 0:cw + 1],
                                        in1=Wt[:, :, 3:cw + 4], op=MIN)
                T2 = pool.tile([NPD, K, cw], f16, tag=f"T2{ch}")
                nc.vector.tensor_tensor(out=T2, in0=T1[:, :, 0:cw],
                                        in1=T1[:, :, 1:cw + 1], op=MIN)

                for (b0, b1, r, psplit) in H_PIECES[ch]:
                    pw = b1 - b0
                    Hm = pool.tile([NPD, K, pw], f16, tag=f"H{ch}_{b0}")
                    nc.vector.tensor_tensor(
                        out=Hm, in0=T2[:, :, b0:b1],
                        in1=Wt[:, :, b0 + 2:b1 + 2], op=MIN)
                    if psplit:
                        half = NPD // 2
                        for (p0, p1, rr) in ((0, half, r),
                                             (half, NPD, 1 - r)):
                            dst = bass.AP(
                                tensor=ys,
                                offset=out_off + p0 * K * pw,
                                ap=[[K * pw, p1 - p0], [1, K * pw]],
                            )
                            rings[rr].dma_start(out=dst, in_=Hm[p0:p1])
                    else:
                        dst = bass.AP(
                            tensor=ys,
                            offset=out_off,
                            ap=[[K * pw, NPD], [1, K * pw]],
                        )
                        rings[r].dma_start(out=dst, in_=Hm)
                    out_off += NPD * K * pw
    nc.compile()
    return nc


def _get_program():
    if "nc" not in _cached:
        _cached["nc"] = _build_program()
    return _cached["nc"]


# stripe gather index: padded-row index per (j, s)
_ROW_IDX = (K * np.arange(PPI)[:, None] + np.arange(SLOTS)[None, :])


def _stripe_core_input(x3: np.ndarray) -> np.ndarray:
    """[3,512,512] f16 -> host-striped flat input (chunk-blocked)."""
    xp = np.zeros((IMGS, PAD_H, PAD_W), np.float16)
    xp[:, 2:2 + H, 2:2 + W] = x3
    stripes = xp[:, _ROW_IDX, :].reshape(NPD, SLOTS, PAD_W)
    parts = [
        np.ascontiguousarray(stripes[:, :, c0:c0 + lw]).reshape(-1)
        for (c0, _), lw in zip(CHUNKS, LWS)
    ]
    return np.concatenate(parts)


def _out_pieces():
    pieces = []
    for ch, (c0, c1) in enumerate(CHUNKS):
        for (b0, b1, _r, _ps) in H_PIECES[ch]:
            pieces.append((c0 + b0, b1 - b0))
    return pieces


_PIECES = None


def _unstripe_core_output(flat: np.ndarray) -> np.ndarray:
    """piece-blocked f16 output -> [3,512,512] f32."""
    global _PIECES
    if _PIECES is None:
        _PIECES = _out_pieces()
    stripes = np.empty((NPD, K, W), np.float16)
    off = 0
    for col0, pw in _PIECES:
        blk = flat[off:off + NPD * K * pw].reshape(NPD, K, pw)
        stripes[:, :, col0:col0 + pw] = blk
        off += NPD * K * pw
    ys = stripes.reshape(IMGS, PPI, K, W)
    out = np.empty((IMGS, H, W), np.float32)
    full = (PPI - 1) * K  # 507 rows from full partitions
    out[:, :full] = ys[:, :PPI - 1].reshape(IMGS, full, W)
    out[:, full:] = ys[:, PPI - 1, :H - full]
    return out


def _run_on_hw(x24: np.ndarray, trace: bool = False):
    from concourse.bass_utils import run_bass_kernel_spmd
    nc = _get_program()
    x24 = x24.astype(np.float16)
    in_maps = [
        {"xs": _stripe_core_input(x24[IMGS * k:IMGS * (k + 1)])}
        for k in range(N_CORES)
    ]
    try:
        res = run_bass_kernel_spmd(nc, in_maps, list(range(N_CORES)),
                                   trace=trace)
    except Exception:
        import time
        time.sleep(5)
        res = run_bass_kernel_spmd(nc, in_maps, list(range(N_CORES)),
                                   trace=trace)
    out = np.stack([
        _unstripe_core_output(res.results[k]["ys"]) for k in range(N_CORES)
    ])
    return out.reshape(B, C, H, W), res


def _erode_reference_np(x: np.ndarray, se: np.ndarray) -> np.ndarray:
    """Generic fallback faithful to the kornia-style formula (numpy)."""
    kh, kw = se.shape
    ph, pw = kh // 2, kw // 2
    xpad = np.pad(x, ((0, 0), (0, 0), (ph, ph), (pw, pw)))
    out = None
    for r in range(kh):
        for c in range(kw):
            shifted = xpad[:, :, r:r + x.shape[2], c:c + x.shape[3]]
            bias = se[r, c] - 1.0
            val = shifted - bias if bias >= 0.0 else np.full_like(shifted, -bias)
            out = val if out is None else np.minimum(out, val)
    return out.astype(x.dtype)


def kernel(x, se):
    x = np.asarray(x, dtype=np.float32)
    se = np.asarray(se, dtype=np.float32)
    if se.shape != (5, 5) or not np.all(se == 1.0) or x.shape != (B, C, H, W):
        return _erode_reference_np(x, se)
    x24 = np.ascontiguousarray(x.reshape(B * C, H, W))
    out, _ = _run_on_hw(x24, trace=False)
    return out
